# revision 1
# baseline (speedup 1.0000x reference)
"""Trainium2 Bass kernel for nn_DTransformer (sparse attention w/ distance decay).

Sharding: data-parallel over batch (bs=8 -> 8 cores, one batch element per
core, weights replicated, no collectives).  Per core the full 3-layer network
runs on-chip.  The dominant cost is the per-element distance-decay pipeline
over [h, s, s] score tiles, organized as (head-group, q-tile) units with tiles
[128 q-partitions, 4 heads, k free]; the suffix attention-mass S comes from a
reversed-AP prefix scan (no cancellation -> bf16-safe), and the second-softmax
row sums come free from a ones-column appended to V in the A@V matmul.
"""

import os
import sys
import contextlib

for _p in ("/opt/trn_rl_repo", "/root/.axon_site/_ro/trn_rl_repo"):
    if os.path.isdir(_p) and _p not in sys.path:
        sys.path.insert(0, _p)

import numpy as np
import ml_dtypes

import concourse.bass as bass
import concourse.mybir as mybir
import concourse.tile as tile
from concourse import bacc

F32 = mybir.dt.float32
F16 = mybir.dt.float16
BF16 = mybir.dt.bfloat16
AF = mybir.ActivationFunctionType
OP = mybir.AluOpType

D = 256
H = 8
HG = 4            # heads per group
NG = H // HG
DK = 32
SEQ = 1024
BS = 8
NQT = SEQ // 128
ISQ = float(1.0 / np.sqrt(np.float32(DK)))
MASKNEG = -1e9
EPS = 1e-5

bf16 = ml_dtypes.bfloat16
KEEP0 = frozenset({0})


def _opt(ap):
    return ap.opt(keep_dims=KEEP0)


def _rev(ap):
    """Reverse the innermost free dim of an AP (squeeze count-1 dims)."""
    pairs = [list(x) for x in ap.ap]
    keep = [pairs[0]] + [x for x in pairs[1:] if x[1] != 1]
    assert len(keep) == 2, f"need 2D-able ap, got {ap.ap}"
    (ps, pc), (fs, fc) = keep
    return bass.AP(tensor=ap.tensor, offset=ap.offset + fs * (fc - 1),
                   ap=[[ps, pc], [-fs, fc]])


def _bc(ap, n):
    """Append a broadcast innermost free dim of size n."""
    pairs = [list(x) for x in ap.ap]
    return bass.AP(tensor=ap.tensor, offset=ap.offset, ap=pairs + [[0, n]])


# ---------------------------------------------------------------- host prep

def host_prep(inputs):
    g = {k: np.asarray(v) for k, v in inputs.items()}

    def f32(x):
        return np.ascontiguousarray(np.asarray(x, dtype=np.float32))

    drv = {}
    for i, names in ((1, ("q", "v", "o")), (2, ("q", "v", "o")),
                     (3, ("k", "v", "o"))):
        for n in names:
            drv[f"WT_{n}{i}"] = f32(g[f"W{n}{i}"].T)      # [din, dout]
    for nm in ("bq1", "bq2", "bk3"):
        drv[nm + "_c"] = f32(np.asarray(g[nm], np.float32).reshape(2, 128).T)
    for nm in ("bv1", "bv2", "bv3", "bo1", "bo2", "bo3", "blv"):
        drv[nm + "_r"] = f32(g[nm]).reshape(1, D)
    drv["blv_b"] = f32(g["blv"]).reshape(1, D)
    for i in (1, 2, 3):
        drv[f"lng{i}_r"] = f32(g[f"lng{i}"]).reshape(1, D)
        drv[f"lnb{i}_r"] = f32(g[f"lnb{i}"]).reshape(1, D)
        gam = -np.logaddexp(0.0, f32(g[f"g{i}"]).reshape(H))
        drv[f"gam2_{i}"] = f32((gam * gam).reshape(1, H))
    know = f32(g["know"]).reshape(D)
    q3 = know @ f32(g["Wq3"]).T + f32(g["bq3"])
    q3blk = np.zeros((D, H), np.float32)
    for h in range(H):
        q3blk[h * DK:(h + 1) * DK, h] = q3[h * DK:(h + 1) * DK]
    drv["q3blk"] = q3blk
    drv["know_r"] = know.reshape(1, D)
    kk = know.reshape(H, DK) @ f32(g["Wlk"]).T + f32(g["blk"])
    kk = 1.0 / (1.0 + np.exp(-kk))
    drv["kkT"] = f32(kk.T)                                # [256, 8]
    drv["WlvT"] = f32(np.tile(g["Wlv"].T, (4, 1)))        # [128, 256] x4 row groups
    p = np.arange(128)[:, None]
    j = np.arange(128)[None, :]
    pos = np.concatenate(
        [np.abs((7 - ob) * 128 + p - j).astype(np.float32) for ob in range(8)],
        axis=1)
    drv["REVPOS"] = np.ascontiguousarray(pos.astype(bf16))
    # mask constants: M0 is added to RAW scores (later scaled by 1/sqrt(dk)):
    # -3e5 * ISQ = -5.3e4 fits fp16.  M3 is added to PRE-scaled layer-3 row
    # scores: -6e4 fits fp16/bf16.  Both make exp() underflow to exactly 0.
    drv["M0"] = f32(np.where(j <= p, 0.0, -3e5))          # inclusive causal
    drv["M3"] = np.ascontiguousarray(
        np.where(j < p, 0.0, -6e4).astype(bf16))          # strict causal
    drv["IDF"] = f32(np.eye(128))
    drv["ONES4"] = f32(np.ones((128, 128)))
    drv["IDB"] = np.ascontiguousarray(np.eye(128).astype(bf16))
    return drv


# ---------------------------------------------------------------- builder

class KB:
    def __init__(self, nc, tc, ctx):
        self.nc, self.tc, self.ctx = nc, tc, ctx

    def pst(self, shape):
        """Shared small PSUM scratch (single tag, <=512 f32 per partition)."""
        return self.pps.tile(shape, F32, tag="ps", name="ps")

    def load_consts(self, dd):
        nc = self.nc
        pool = self.ctx.enter_context(self.tc.tile_pool(name="consts", bufs=1))
        sb = {}
        for i, names in ((1, ("q", "v", "o")), (2, ("q", "v", "o")),
                         (3, ("k", "v", "o"))):
            for n in names:
                t = pool.tile([128, 2, D], F32, tag=f"WT_{n}{i}")
                nc.sync.dma_start(
                    out=t[:],
                    in_=dd[f"WT_{n}{i}"][:].rearrange("(a p) d -> p a d", p=128))
                sb[f"WT_{n}{i}"] = t
        for nm in ("q3blk", "kkT"):
            t = pool.tile([128, 2, H], F32, tag=nm)
            nc.sync.dma_start(
                out=t[:], in_=dd[nm][:].rearrange("(a p) h -> p a h", p=128))
            sb[nm] = t
        for nm in ("bq1_c", "bq2_c", "bk3_c", "WlvT", "REVPOS", "M0", "M3",
                   "IDF", "IDB", "bo1_r", "bo2_r", "bo3_r", "blv_r"):
            src = dd[nm]
            t = pool.tile(list(src.shape), src.dtype, tag=nm)
            nc.sync.dma_start(out=t[:], in_=src[:])
            sb[nm] = t
        for nm in ("ONES4",):
            src = dd[nm]
            t = pool.tile(list(src.shape), src.dtype, tag=nm)
            nc.sync.dma_start(out=t[:], in_=src[:])
            sb[nm] = t
        for nm in ("bv1_r", "bv2_r", "bv3_r", "lng1_r", "lng2_r", "lng3_r",
                   "lnb1_r", "lnb2_r", "lnb3_r", "know_r", "gam2_1", "gam2_2",
                   "gam2_3", "blv_b"):
            src = dd[nm]
            n = src.shape[1]
            t = pool.tile([128, n], F32, tag=nm)
            nc.sync.dma_start(
                out=t[:],
                in_=bass.AP(tensor=src, offset=0, ap=[[0, 128], [1, n]]))
            sb[nm] = t
        ones = pool.tile([1, 128], F32, tag="ones")
        nc.vector.memset(ones[:], 1.0)
        sb["ones"] = ones
        epst = pool.tile([128, 1], F32, tag="eps")
        nc.vector.memset(epst[:], EPS)
        sb["eps"] = epst
        self.sb = sb
        # pre-touch identities on PE so later transposes carry a single
        # DMA-queue wait (walrus allows only one sync wait on LDWEIGHTS)
        junk = pool.tile([128, 2], F32, tag="junk")
        wf = self.pps.tile([128, 128], F32, tag="ps", name="warmf")
        nc.tensor.transpose(wf[:], sb["IDF"][:], sb["IDF"][:])
        nc.scalar.copy(out=junk[:, 0:1], in_=wf[:, 0:1])
        wb = self.pps.tile([128, 128], BF16, tag="ps", name="warmb")
        nc.tensor.transpose(wb[:], sb["IDB"][:], sb["IDB"][:])
        nc.scalar.copy(out=junk[:, 1:2], in_=wb[:, 0:1])

    def hslice(self, T, h, cols):
        """Head-rows slice of a [128, 2, SEQ] transposed tensor: [32, len]."""
        return _opt(T[(h % 4) * DK:(h % 4 + 1) * DK, h // 4, cols])

    def load_nat(self, dram, pool, tag):
        tiles = []
        for st in range(NQT):
            t = pool.tile([128, D], F32, tag=f"{tag}{st}")
            self.nc.sync.dma_start(out=t[:],
                                   in_=dram[st * 128:(st + 1) * 128, :])
            tiles.append(t)
        return tiles

    def transpose_nat(self, x_tiles, pool, tag):
        """natural [8][128,256] (tiles or APs) -> [128, 2, 1024] f32."""
        nc = self.nc
        xT = pool.tile([128, 2, SEQ], F32, tag=tag)
        for st in range(NQT):
            ps = self.pst([128, 2, 128])
            for dh in range(2):
                nc.tensor.transpose(_opt(ps[:, dh, :]),
                                    _opt(x_tiles[st][:, dh * 128:(dh + 1) * 128]),
                                    self.sb["IDF"][:])
            nc.scalar.copy(out=_opt(xT[:, :, st * 128:(st + 1) * 128]),
                           in_=ps[:])
        return xT

    def proj_T(self, xT, wname, bname, pool, tag):
        """out[do, s] = W @ x.T + b : [128, 2, 1024] f32."""
        nc = self.nc
        W = self.sb[wname]
        out = pool.tile([128, 2, SEQ], F32, tag=tag)
        for dh in range(2):
            for sc in range(2):
                ps = self.pst([128, 512])
                for ih in range(2):
                    nc.tensor.matmul(
                        ps[:], _opt(W[:, ih, dh * 128:(dh + 1) * 128]),
                        _opt(xT[:, ih, sc * 512:(sc + 1) * 512]),
                        start=(ih == 0), stop=(ih == 1))
                nc.scalar.activation(
                    out=_opt(out[:, dh, sc * 512:(sc + 1) * 512]), in_=ps[:],
                    func=AF.Identity, bias=self.sb[bname][:, dh:dh + 1],
                    scale=1.0)
        return out

    def proj_V(self, xT, wname, bname, pool, tag):
        """V natural with ones column: [8][128, H, 33] bf16."""
        nc = self.nc
        W = self.sb[wname]
        bias = self.sb[bname]
        tiles = []
        for st in range(NQT):
            ps = self.pst([128, D])
            for ih in range(2):
                nc.tensor.matmul(ps[:],
                                 _opt(xT[:, ih, st * 128:(st + 1) * 128]),
                                 _opt(W[:, ih, :]),
                                 start=(ih == 0), stop=(ih == 1))
            v = pool.tile([128, H, DK + 1], BF16, tag=f"{tag}{st}")
            nc.vector.tensor_tensor(
                out=v[:, :, 0:DK],
                in0=ps[:].rearrange("p (h d) -> p h d", h=H),
                in1=bias[:].rearrange("p (h d) -> p h d", h=H), op=OP.add)
            nc.vector.memset(_opt(v[:, :, DK:DK + 1]), 1.0)
            tiles.append(v)
        return tiles

    # ------------------------------------------------ attention (one layer)
    def attention(self, lay, QT, KT, V, attn_pool, c3bc=None, ncum3bc=None,
                  cum3pad=None):
        nc, sb, tc = self.nc, self.sb, self.tc
        strict = (lay == 3)
        with contextlib.ExitStack() as actx:
            if not strict:
                zq = actx.enter_context(
                    tc.tile_pool(name=f"zq{lay}", bufs=2, space="PSUM"))
            wtq = actx.enter_context(
                tc.tile_pool(name=f"wtq{lay}", bufs=1 if not strict else 2,
                             space="PSUM"))
            oq = actx.enter_context(
                tc.tile_pool(name=f"oq{lay}", bufs=1 if not strict else 2,
                             space="PSUM"))
            rowp = actx.enter_context(tc.tile_pool(name=f"row{lay}", bufs=2))
            zsp = actx.enter_context(tc.tile_pool(name=f"zsp{lay}", bufs=3))
            chn = actx.enter_context(tc.tile_pool(name=f"chn{lay}", bufs=2))
            sml = actx.enter_context(tc.tile_pool(name=f"sml{lay}", bufs=4))
            wts = actx.enter_context(tc.tile_pool(name=f"wts{lay}", bufs=5))

            attn = []
            for qt in range(NQT):
                Kt = (qt + 1) * 128
                rp_ap = _opt(sb["REVPOS"][:, (7 - qt) * 128:
                                          (7 - qt) * 128 + Kt])
                at = attn_pool.tile([128, H, DK], F32, tag=f"attn{lay}_{qt}")
                G = {}
                for hg in range(NG):
                    g = {}
                    g["u"] = chn.tile([128, HG, Kt], BF16, tag="u", name="u")
                    g["t"] = rowp.tile([128, HG, Kt], F16, tag="t", name="t")
                    g["w"] = rowp.tile([128, HG, Kt], BF16, tag="w", name="w")
                    g["o"] = oq.tile([128, HG, DK + 1], F32, tag="o", name="o")
                    if not strict:
                        g["e"] = rowp.tile([128, HG, Kt], BF16, tag="e",
                                           name="e")
                        g["S"] = rowp.tile([128, HG, Kt], BF16, tag="S",
                                           name="S")
                        g["zs"] = zsp.tile([128, HG, Kt], F16, tag="zs",
                                           name="zs")
                        g["E"] = sml.tile([128, HG], F32, tag="E", name="E")
                    g["rEg"] = sml.tile([128, HG], F32, tag="rEg", name="rEg")
                    G[hg] = g
                if strict:
                    # E3 column for this q-tile: cum3pad[h, q] transposed
                    e3ps = self.pst([128, 8])
                    nc.tensor.transpose(
                        e3ps[:], _opt(cum3pad[:, qt * 128:qt * 128 + 128]),
                        _opt(sb["IDF"][0:8, 0:8]))
                    E3 = sml.tile([128, H], F32, tag="E3", name="E3")
                    nc.vector.tensor_scalar_max(out=E3[:], in0=e3ps[:],
                                                scalar1=1e-30)
                    rE3 = sml.tile([128, H], F32, tag="rE3", name="rE3")
                    nc.vector.reciprocal(out=rE3[:], in_=E3[:])
                    nc.vector.tensor_tensor(out=rE3[:], in0=rE3[:],
                                            in1=sb["gam2_3"][:], op=OP.mult)

                if not strict:
                    # ---- scores -> PSUM -> scaled fp16 copy (both groups)
                    for hg in range(NG):
                        g = G[hg]
                        for i, h in enumerate(range(hg * HG, hg * HG + HG)):
                            z = zq.tile([128, Kt], F32, tag="z", name="z")
                            lhs = self.hslice(
                                QT, h, slice(qt * 128, qt * 128 + 128))
                            tp = ((h % 4) * DK, 0)
                            nchunk = (Kt + 511) // 512
                            for ci in range(nchunk):
                                kc = ci * 512
                                cl = min(512, Kt - kc)
                                nc.tensor.matmul(
                                    _opt(z[:, kc:kc + cl]), lhs,
                                    self.hslice(KT, h, slice(kc, kc + cl)),
                                    start=True, stop=False, tile_position=tp,
                                    skip_group_check=True)
                            nc.tensor.matmul(
                                _opt(z[:, Kt - 128:Kt]), sb["IDF"][:],
                                sb["M0"][:], start=False, stop=True,
                                skip_group_check=True)
                            if i % 2 == 0:
                                nc.vector.tensor_scalar_mul(
                                    out=_opt(g["zs"][:, i, :]), in0=z[:],
                                    scalar1=ISQ)
                            else:
                                nc.scalar.mul(
                                    out=_opt(g["zs"][:, i, :]), in_=z[:],
                                    mul=ISQ)
                    # ---- e = exp(zs)
                    for hg in range(NG):
                        nc.scalar.activation(out=G[hg]["e"][:],
                                             in_=G[hg]["zs"][:], func=AF.Exp)
                    # ---- reversed scan -> exclusive suffix sums S
                    for hg in range(NG):
                        g = G[hg]
                        for i in range(HG):
                            nc.vector.tensor_tensor_scan(
                                out=_rev(g["S"][:, i, 0:Kt - 1]),
                                data0=_rev(g["e"][:, i, 1:Kt]),
                                data1=_rev(g["e"][:, i, 1:Kt]),
                                initial=0.0, op0=OP.add, op1=OP.bypass)
                            nc.vector.memset(_opt(g["S"][:, i, Kt - 1:Kt]),
                                             0.0)
                    # ---- E = S[0] + e[0];  rEg = gamma^2 / E
                    for hg in range(NG):
                        g = G[hg]
                        nc.vector.tensor_tensor(
                            out=g["E"][:], in0=_opt(g["S"][:, :, 0:1]),
                            in1=_opt(g["e"][:, :, 0:1]), op=OP.add)
                        nc.vector.reciprocal(out=g["rEg"][:], in_=g["E"][:])
                        nc.vector.tensor_tensor(
                            out=g["rEg"][:], in0=g["rEg"][:],
                            in1=_opt(sb[f"gam2_{lay}"][:, hg * HG:
                                                       hg * HG + HG]),
                            op=OP.mult)
                    # ---- u = (S * rEg_h) * pos
                    for hg in range(NG):
                        g = G[hg]
                        for i in range(HG):
                            nc.vector.scalar_tensor_tensor(
                                out=_opt(g["u"][:, i, :]),
                                in0=_opt(g["S"][:, i, :]),
                                scalar=g["rEg"][:, i:i + 1], in1=rp_ap,
                                op0=OP.mult, op1=OP.mult)
                    # ---- d = sqrt(u) in place; eff = exp(-d) in place
                    for hg in range(NG):
                        nc.scalar.activation(out=G[hg]["u"][:],
                                             in_=G[hg]["u"][:], func=AF.Sqrt)
                    for hg in range(NG):
                        nc.scalar.activation(out=G[hg]["u"][:],
                                             in_=G[hg]["u"][:], func=AF.Exp,
                                             scale=-1.0)
                    # ---- t = max(eff, 1e-5) * zs
                    for hg in range(NG):
                        g = G[hg]
                        for i in range(HG):
                            nc.vector.scalar_tensor_tensor(
                                out=_opt(g["t"][:, i, :]),
                                in0=_opt(g["u"][:, i, :]), scalar=1e-5,
                                in1=_opt(g["zs"][:, i, :]),
                                op0=OP.max, op1=OP.mult)
                else:
                    # ---- layer 3: u = (E3 + (-cum)) * pos, clamp diag
                    for hg in range(NG):
                        g = G[hg]
                        for i, h in enumerate(range(hg * HG, hg * HG + HG)):
                            nc.vector.scalar_tensor_tensor(
                                out=_opt(g["u"][:, i, :]),
                                in0=_opt(ncum3bc[h][:, 0:Kt]),
                                scalar=_opt(E3[:, h:h + 1]),
                                in1=rp_ap, op0=OP.add, op1=OP.mult)
                            nc.vector.tensor_scalar_max(
                                out=_opt(g["u"][:, i, Kt - 128:Kt]),
                                in0=_opt(g["u"][:, i, Kt - 128:Kt]),
                                scalar1=0.0)
                    # ---- d = sqrt(u * gamma^2/E) per head (scale AP)
                    for hg in range(NG):
                        g = G[hg]
                        for i, h in enumerate(range(hg * HG, hg * HG + HG)):
                            nc.scalar.activation(
                                out=_opt(g["u"][:, i, :]),
                                in_=_opt(g["u"][:, i, :]),
                                func=AF.Sqrt, scale=_opt(rE3[:, h:h + 1]))
                    for hg in range(NG):
                        nc.scalar.activation(out=G[hg]["u"][:],
                                             in_=G[hg]["u"][:], func=AF.Exp,
                                             scale=-1.0)
                    # ---- t = max(eff, 1e-5) * c3 (masked diag)
                    for hg in range(NG):
                        g = G[hg]
                        for i, h in enumerate(range(hg * HG, hg * HG + HG)):
                            if qt > 0:
                                nc.vector.scalar_tensor_tensor(
                                    out=_opt(g["t"][:, i, 0:Kt - 128]),
                                    in0=_opt(g["u"][:, i, 0:Kt - 128]),
                                    scalar=1e-5,
                                    in1=_opt(c3bc[h][:, 0:Kt - 128]),
                                    op0=OP.max, op1=OP.mult)
                            c3m = sml.tile([128, 128], BF16, tag="c3m",
                                           name="c3m")
                            nc.vector.tensor_tensor(
                                out=c3m[:], in0=_opt(c3bc[h][:, Kt - 128:Kt]),
                                in1=sb["M3"][:], op=OP.add)
                            nc.vector.scalar_tensor_tensor(
                                out=_opt(g["t"][:, i, Kt - 128:Kt]),
                                in0=_opt(g["u"][:, i, Kt - 128:Kt]),
                                scalar=1e-5, in1=c3m[:],
                                op0=OP.max, op1=OP.mult)
                # ---- w = exp(t)
                for hg in range(NG):
                    nc.scalar.activation(out=G[hg]["w"][:], in_=G[hg]["t"][:],
                                         func=AF.Exp)
                # ---- transpose w via PE; A @ [V | 1] accumulation
                for hg in range(NG):
                    g = G[hg]
                    for i, h in enumerate(range(hg * HG, hg * HG + HG)):
                        nblk = qt + 1
                        for g0 in range(0, nblk, 4):
                            gl = min(4, nblk - g0)
                            wt_ps = wtq.tile([128, 512], BF16, tag="wt",
                                             name="wt")
                            for j in range(gl):
                                kb = g0 + j
                                nc.tensor.transpose(
                                    _opt(wt_ps[:, j * 128:(j + 1) * 128]),
                                    _opt(g["w"][:, i,
                                                kb * 128:(kb + 1) * 128]),
                                    sb["IDB"][:])
                            wt_sb = wts.tile([128, 512], BF16, tag="wts",
                                             name="wts")
                            if (i + g0 // 4) % 2 == 0:
                                nc.vector.tensor_copy(
                                    out=_opt(wt_sb[:, 0:gl * 128]),
                                    in_=_opt(wt_ps[:, 0:gl * 128]))
                            else:
                                nc.scalar.copy(
                                    out=_opt(wt_sb[:, 0:gl * 128]),
                                    in_=_opt(wt_ps[:, 0:gl * 128]))
                            for j in range(gl):
                                kb = g0 + j
                                nc.tensor.matmul(
                                    _opt(g["o"][:, i, :]),
                                    _opt(wt_sb[:, j * 128:(j + 1) * 128]),
                                    _opt(V[kb][:, h, :]),
                                    start=(kb == 0), stop=(kb == qt),
                                    skip_group_check=True)
                # ---- normalize attn = o / W
                for hg in range(NG):
                    g = G[hg]
                    Wg = sml.tile([128, HG], F32, tag="Wg", name="Wg")
                    rW = sml.tile([128, HG], F32, tag="rW", name="rW")
                    nc.vector.tensor_scalar_max(
                        out=Wg[:], in0=_opt(g["o"][:, :, DK:DK + 1]),
                        scalar1=1e-30)
                    nc.vector.reciprocal(out=rW[:], in_=Wg[:])
                    nc.vector.tensor_tensor(
                        out=_opt(at[:, hg * HG:hg * HG + HG, :]),
                        in0=_opt(g["o"][:, :, 0:DK]), in1=_bc(rW[:], DK),
                        op=OP.mult)
                attn.append(at)
            return attn

    # ------------------------------------------------ out proj + LN
    def out_ln(self, lay, attn, res_tiles, hpool, spool, tpool,
               dram_out=None):
        nc, sb = self.nc, self.sb
        attnT = self.transpose_nat(
            [a[:].rearrange("p h d -> p (h d)") for a in attn],
            tpool, tag=f"attnT{lay}")
        W = sb[f"WT_o{lay}"]
        out_tiles = []
        for st in range(NQT):
            ps = self.pst([128, D])
            for ih in range(2):
                nc.tensor.matmul(ps[:],
                                 _opt(attnT[:, ih, st * 128:(st + 1) * 128]),
                                 _opt(W[:, ih, :]), start=(ih == 0), stop=False)
            nc.tensor.matmul(ps[:], sb["ones"][:], sb[f"bo{lay}_r"][:],
                             start=False, stop=True)
            res = res_tiles[st] if isinstance(res_tiles, list) else res_tiles
            x = spool.tile([128, D], F32, tag="lnx")
            nc.vector.tensor_tensor(out=x[:], in0=ps[:], in1=res[:], op=OP.add)
            stats = spool.tile([128, 6], F32, tag="bnst")
            mv = spool.tile([128, 2], F32, tag="bnmv")
            nc.vector.bn_stats(out=stats[:], in_=x[:])
            nc.vector.bn_aggr(out=mv[:], in_=stats[:])
            sd = spool.tile([128, 1], F32, tag="sd")
            nc.scalar.activation(out=sd[:], in_=_opt(mv[:, 1:2]), func=AF.Sqrt,
                                 bias=sb["eps"][:], scale=1.0)
            rstd = spool.tile([128, 1], F32, tag="rstd")
            nc.vector.reciprocal(out=rstd[:], in_=sd[:])
            xn = spool.tile([128, D], F32, tag="lnxn")
            nc.vector.tensor_scalar(
                out=xn[:], in0=x[:], scalar1=_opt(mv[:, 0:1]), scalar2=rstd[:],
                op0=OP.subtract, op1=OP.mult)
            if dram_out is None:
                ho = hpool.tile([128, D], F32, tag=f"h{lay}_{st}")
            else:
                ho = spool.tile([128, D], F32, tag="ho", name="ho")
            nc.vector.tensor_tensor(out=ho[:], in0=xn[:],
                                    in1=sb[f"lng{lay}_r"][:], op=OP.mult)
            nc.vector.tensor_tensor(out=ho[:], in0=ho[:],
                                    in1=sb[f"lnb{lay}_r"][:], op=OP.add)
            if dram_out is not None:
                nc.sync.dma_start(out=dram_out[st * 128:(st + 1) * 128, :],
                                  in_=ho[:])
            out_tiles.append(ho)
        return out_tiles

    # ------------------------------------------------ layer-3 prologue
    def l3_rows(self, h1T, mpool, lpool):
        nc, sb = self.nc, self.sb
        KT3 = self.proj_T(h1T, "WT_k3", "bk3_c", mpool, tag="KT3")
        c3 = lpool.tile([8, SEQ], F32, tag="c3")
        for sc in range(2):
            ps = self.pst([8, 512])
            for ih in range(2):
                nc.tensor.matmul(ps[:], _opt(sb["q3blk"][:, ih, :]),
                                 _opt(KT3[:, ih, sc * 512:(sc + 1) * 512]),
                                 start=(ih == 0), stop=(ih == 1))
            nc.vector.tensor_scalar_mul(
                out=_opt(c3[:, sc * 512:(sc + 1) * 512]), in0=ps[:],
                scalar1=ISQ)
        e3 = lpool.tile([8, SEQ], F32, tag="e3")
        nc.scalar.activation(out=e3[:], in_=c3[:], func=AF.Exp)
        cum3pad = lpool.tile([8, SEQ + 128], F32, tag="cum3pad")
        nc.vector.memset(_opt(cum3pad[:, 0:1]), 0.0)
        nc.vector.tensor_tensor_scan(
            out=_opt(cum3pad[:, 1:SEQ + 1]), data0=e3[:], data1=e3[:],
            initial=0.0, op0=OP.add, op1=OP.bypass)
        nc.vector.memset(_opt(cum3pad[:, SEQ + 1:]), 0.0)
        c3bc, ncum3bc = [], []
        for h in range(H):
            cb = lpool.tile([128, SEQ], BF16, tag=f"c3bc{h}")
            ncb = lpool.tile([128, SEQ], F32, tag=f"ncum{h}")
            for sc in range(2):
                # stage rows at partition 0 (matmul rhs needs base part 0)
                stc = mpool.tile([1, 512], F32, tag="stc", name="stc")
                stn = mpool.tile([1, 512], F32, tag="stn", name="stn")
                nc.sync.dma_start(
                    out=stc[:], in_=_opt(c3[h:h + 1, sc * 512:(sc + 1) * 512]))
                nc.sync.dma_start(
                    out=stn[:], in_=_opt(cum3pad[h:h + 1,
                                                 sc * 512 + 1:sc * 512 + 513]))
                ps = self.pst([128, 512])
                nc.tensor.matmul(ps[:], sb["ones"][:], stc[:],
                                 start=True, stop=True)
                nc.scalar.copy(out=_opt(cb[:, sc * 512:(sc + 1) * 512]),
                               in_=ps[:])
                ps2 = self.pst([128, 512])
                nc.tensor.matmul(ps2[:], sb["ones"][:], stn[:],
                                 start=True, stop=True)
                nc.vector.tensor_scalar_mul(
                    out=_opt(ncb[:, sc * 512:(sc + 1) * 512]), in0=ps2[:],
                    scalar1=-1.0)
            c3bc.append(cb)
            ncum3bc.append(ncb)
        return c3bc, ncum3bc, cum3pad

    # ------------------------------------------------ final mixture
    def final(self, hh, xT_q, out_dram, spool, tpool):
        nc, sb = self.nc, self.sb
        hhT = self.transpose_nat(hh, tpool, tag="hhT")
        als = []
        for st in range(NQT):
            bps = self.pst([128, H])
            for ih in range(2):
                nc.tensor.matmul(bps[:],
                                 _opt(xT_q[:, ih, st * 128:(st + 1) * 128]),
                                 _opt(sb["kkT"][:, ih, :]),
                                 start=(ih == 0), stop=(ih == 1))
            nmax = spool.tile([128, 1], F32, tag="nmax", name="nmax")
            nc.vector.tensor_reduce(out=nmax[:], in_=bps[:],
                                    axis=mybir.AxisListType.X, op=OP.max,
                                    negate=True)
            au = spool.tile([128, H], F32, tag="au", name="au")
            sa = spool.tile([128, 1], F32, tag="sa", name="sa")
            nc.scalar.activation(out=au[:], in_=bps[:], func=AF.Exp,
                                 bias=nmax[:], scale=1.0, accum_out=sa[:])
            rsa = spool.tile([128, 1], F32, tag="rsa", name="rsa")
            nc.vector.reciprocal(out=rsa[:], in_=sa[:])
            al = tpool.tile([128, H], F32, tag=f"al{st}", name="al")
            nc.vector.tensor_scalar_mul(out=al[:], in0=au[:], scalar1=rsa[:])
            als.append(al)
        for st in range(NQT):
            al = als[st]
            acc = spool.tile([128, D], F32, tag="facc", name="facc")
            for h in range(H):
                vps = self.pst([128, D])
                nc.tensor.matmul(
                    vps[:],
                    self.hslice(hhT, h, slice(st * 128, st * 128 + 128)),
                    _opt(sb["WlvT"][(h % 4) * DK:(h % 4 + 1) * DK, :]),
                    start=True, stop=False,
                    tile_position=((h % 4) * DK, 0), skip_group_check=True)
                o = (h % 4) * DK
                nc.tensor.matmul(
                    vps[:], _opt(sb["ONES4"][o:o + 1, :]),
                    _opt(sb["blv_b"][o:o + 1, :]), start=False, stop=True,
                    tile_position=(o, 0), skip_group_check=True)
                vsb = spool.tile([128, D], BF16, tag="vsb", name="vsb")
                nc.scalar.activation(out=vsb[:], in_=vps[:], func=AF.Sigmoid)
                if h == 0:
                    nc.vector.tensor_scalar_mul(out=acc[:], in0=vsb[:],
                                                scalar1=al[:, 0:1])
                else:
                    nc.vector.scalar_tensor_tensor(
                        out=acc[:], in0=vsb[:], scalar=al[:, h:h + 1],
                        in1=acc[:], op0=OP.mult, op1=OP.add)
            nc.sync.dma_start(out=out_dram[st * 128:(st + 1) * 128, :],
                              in_=acc[:])


def build(derived, debug=False, stop_after=None):
    nc = bacc.Bacc(None, target_bir_lowering=False)
    dd = {}
    for name, arr in derived.items():
        dt = {np.dtype(np.float32): F32, np.dtype(bf16): BF16,
              np.dtype(np.float16): F16}[np.dtype(arr.dtype)]
        dd[name] = nc.dram_tensor(name, list(arr.shape), dt,
                                  kind="ExternalInput")
    x_q = nc.dram_tensor("x_q", [SEQ, D], F32, kind="ExternalInput")
    x_s = nc.dram_tensor("x_s", [SEQ, D], F32, kind="ExternalInput")
    out = nc.dram_tensor("out", [SEQ, D], F32, kind="ExternalOutput")

    def dump(tiles, name):
        if not debug:
            return
        t = nc.dram_tensor(name, [SEQ, D], F32, kind="ExternalOutput")
        for st in range(NQT):
            ap = tiles[st][:]
            if len(ap.shape) == 3:
                ap = ap.rearrange("p h d -> p (h d)")
            nc.sync.dma_start(out=t[st * 128:(st + 1) * 128, :], in_=ap)

    with tile.TileContext(nc) as tc, contextlib.ExitStack() as ctx:
        kb = KB(nc, tc, ctx)
        kb.pps = ctx.enter_context(
            tc.tile_pool(name="pps", bufs=2, space="PSUM"))
        kb.load_consts(dd)
        sb = kb.sb
        glob = ctx.enter_context(tc.tile_pool(name="glob", bufs=1))

        h1d = nc.dram_tensor("dbg_h1", [SEQ, D], F32,
                             kind="ExternalOutput" if debug else "Internal")
        h2d = nc.dram_tensor("dbg_h2", [SEQ, D], F32,
                             kind="ExternalOutput" if debug else "Internal")
        # ---------------- layer 1 (on x_q) ----------------
        with tc.tile_pool(name="r1", bufs=1) as r1, \
                tc.tile_pool(name="r1s", bufs=2) as r1s:
            xq_nat = kb.load_nat(x_q, r1, "xq")
            xT_q = kb.transpose_nat(xq_nat, glob, tag="xTq")
            QT1 = kb.proj_T(xT_q, "WT_q1", "bq1_c", r1, tag="QT1")
            V1 = kb.proj_V(xT_q, "WT_v1", "bv1_r", r1, tag="V1")
            if stop_after == "pre1":
                for st in range(NQT):
                    nc.sync.dma_start(out=out[st * 128:(st + 1) * 128, :],
                                      in_=xq_nat[st][:])
            if stop_after in (None, "l1", "l2", "pro", "l3"):
                attn1 = kb.attention(1, QT1, QT1, V1, r1)
                dump(attn1, "dbg_attn1")
                kb.out_ln(1, attn1, xq_nat, None, r1s, r1, dram_out=h1d)
            if stop_after == "l1":
                h1n0 = kb.load_nat(h1d, r1, "h1o")
                for st in range(NQT):
                    nc.sync.dma_start(out=out[st * 128:(st + 1) * 128, :],
                                      in_=h1n0[st][:])
        # ---------------- layer 2 (on x_s) ----------------
        if stop_after in (None, "l2", "pro", "l3"):
            with tc.tile_pool(name="r2", bufs=1) as r2, \
                    tc.tile_pool(name="r2s", bufs=2) as r2s:
                xs_nat = kb.load_nat(x_s, r2, "xs")
                xT_s = kb.transpose_nat(xs_nat, r2, tag="xTs")
                QT2 = kb.proj_T(xT_s, "WT_q2", "bq2_c", r2, tag="QT2")
                V2 = kb.proj_V(xT_s, "WT_v2", "bv2_r", r2, tag="V2")
                attn2 = kb.attention(2, QT2, QT2, V2, r2)
                kb.out_ln(2, attn2, xs_nat, None, r2s, r2, dram_out=h2d)
                if stop_after == "l2":
                    h2n0 = kb.load_nat(h2d, r2, "h2o")
                    for st in range(NQT):
                        nc.sync.dma_start(out=out[st * 128:(st + 1) * 128, :],
                                          in_=h2n0[st][:])
        # ---------------- layer 3 prologue ----------------
        if stop_after in (None, "pro", "l3"):
            lpool = ctx.enter_context(tc.tile_pool(name="l3pool", bufs=1))
            with tc.tile_pool(name="l3tmp", bufs=1) as l3tmp:
                h1n = kb.load_nat(h1d, l3tmp, "h1n")
                h2n = kb.load_nat(h2d, l3tmp, "h2n")
                h1T = kb.transpose_nat(h1n, l3tmp, tag="h1T")
                h2T = kb.transpose_nat(h2n, l3tmp, tag="h2T")
                V3 = kb.proj_V(h2T, "WT_v3", "bv3_r", lpool, tag="V3")
                c3bc, ncum3bc, cum3pad = kb.l3_rows(h1T, l3tmp, lpool)
                if stop_after == "pro":
                    for st in range(NQT):
                        o32 = l3tmp.tile([128, D], F32, tag=f"o32_{st}")
                        nc.vector.tensor_copy(
                            out=o32[:].rearrange("p (h d) -> p h d", h=H),
                            in_=V3[st][:, :, 0:DK])
                        nc.sync.dma_start(out=out[st * 128:(st + 1) * 128, :],
                                          in_=o32[:])
        # ---------------- layer 3 + final ----------------
        if stop_after in (None, "l3"):
            with tc.tile_pool(name="r3", bufs=1) as r3, \
                    tc.tile_pool(name="r3s", bufs=2) as r3s:
                attn3 = kb.attention(3, None, None, V3, r3, c3bc=c3bc,
                                     ncum3bc=ncum3bc, cum3pad=cum3pad)
                dump(attn3, "dbg_attn3")
                hh = kb.out_ln(3, attn3, sb["know_r"], r3, r3s, r3)
                dump(hh, "dbg_hh")
                if stop_after == "l3":
                    for st in range(NQT):
                        nc.sync.dma_start(out=out[st * 128:(st + 1) * 128, :],
                                          in_=hh[st][:])
                if stop_after is None:
                    kb.final(hh, xT_q, out, r3s, r3)
    nc.compile()
    return nc


_CACHE = {}


def kernel(**inputs):
    drv = host_prep(inputs)
    if "nc" not in _CACHE:
        _CACHE["nc"] = build(drv)
    nc = _CACHE["nc"]
    q = np.ascontiguousarray(np.asarray(inputs["q_emb"], np.float32))
    s = np.ascontiguousarray(np.asarray(inputs["s_emb"], np.float32))
    in_maps = []
    for b in range(BS):
        m = dict(drv)
        m["x_q"] = np.ascontiguousarray(q[b])
        m["x_s"] = np.ascontiguousarray(s[b])
        in_maps.append(m)
    from concourse.bass_utils import run_bass_kernel_spmd
    res = run_bass_kernel_spmd(nc, in_maps, core_ids=list(range(BS)))
    out = np.stack([np.asarray(res.results[b]["out"]) for b in range(BS)],
                   axis=0)
    return out.astype(np.float32)


if __name__ == "__main__":
    print("kernel module loaded OK")



# revision 8
# speedup vs baseline: 1.3750x; 1.3750x over previous
"""Trainium2 Bass kernel for nn_DTransformer (sparse attention w/ distance decay).

Sharding: data-parallel over batch (bs=8 -> 8 cores, one batch element per
core, weights replicated, no collectives).  Per core the full 3-layer network
runs on-chip.  v2: fp16 matmul operands everywhere (1 cyc/row vs 4 for fp32),
DMA-xbar transposes for the attention-weight tiles (frees the PE and kills the
PSUM->SBUF copies), 2x-mode tensor_tensor ops instead of 1x scalar_tensor_tensor,
gamma^2/E folded into the sqrt activation's per-partition scale.
"""

import os
import sys
import contextlib

for _p in ("/opt/trn_rl_repo", "/root/.axon_site/_ro/trn_rl_repo"):
    if os.path.isdir(_p) and _p not in sys.path:
        sys.path.insert(0, _p)

import numpy as np
import ml_dtypes

import concourse.bass as bass
import concourse.mybir as mybir
import concourse.tile as tile
from concourse import bacc

F32 = mybir.dt.float32
F16 = mybir.dt.float16
BF16 = mybir.dt.bfloat16
AF = mybir.ActivationFunctionType
OP = mybir.AluOpType

D = 256
H = 8
HG = 4            # heads per group
NG = H // HG
DK = 32
SEQ = 1024
BS = 8
NQT = SEQ // 128
ISQ = float(1.0 / np.sqrt(np.float32(DK)))
EPS = 1e-5

bf16 = ml_dtypes.bfloat16
f16 = np.float16
KEEP0 = frozenset({0})


def _opt(ap):
    return ap.opt(keep_dims=KEEP0)


def _rev(ap):
    """Reverse the innermost free dim of an AP (squeeze count-1 dims)."""
    pairs = [list(x) for x in ap.ap]
    keep = [pairs[0]] + [x for x in pairs[1:] if x[1] != 1]
    assert len(keep) == 2, f"need 2D-able ap, got {ap.ap}"
    (ps, pc), (fs, fc) = keep
    return bass.AP(tensor=ap.tensor, offset=ap.offset + fs * (fc - 1),
                   ap=[[ps, pc], [-fs, fc]])


def _bc(ap, n):
    """Append a broadcast innermost free dim of size n."""
    pairs = [list(x) for x in ap.ap]
    return bass.AP(tensor=ap.tensor, offset=ap.offset, ap=pairs + [[0, n]])


def _bc_mid(ap, n):
    """[128, F] AP -> [128, n(bcast), F]."""
    pairs = [list(x) for x in ap.ap]
    assert len(pairs) == 2
    return bass.AP(tensor=ap.tensor, offset=ap.offset,
                   ap=[pairs[0], [0, n], pairs[1]])


def _prow(t, n):
    """Broadcast a [1, n] dram row across 128 partitions (for DMA)."""
    return bass.AP(tensor=t, offset=0, ap=[[0, 128], [1, n]])


# ---------------------------------------------------------------- host prep

def host_prep(inputs):
    g = {k: np.asarray(v) for k, v in inputs.items()}

    def f32(x):
        return np.ascontiguousarray(np.asarray(x, dtype=np.float32))

    def h16(x):
        return np.ascontiguousarray(np.asarray(x, dtype=np.float32).astype(f16))

    drv = {}
    # layers 1/2 share K==Q, so fold sqrt(ISQ) into Wq (applied twice in QK^T)
    SQI = float(np.sqrt(ISQ))
    for i, names in ((1, ("q", "v", "o")), (2, ("q", "v", "o")),
                     (3, ("k", "v", "o"))):
        for n in names:
            w = f32(g[f"W{n}{i}"].T)                     # [din, dout]
            if n == "q":
                w = w * SQI
            drv[f"WT_{n}{i}"] = h16(w)
    for nm in ("bq1", "bq2"):
        drv[nm + "_c"] = f32((np.asarray(g[nm], np.float32) * SQI)
                             .reshape(2, 128).T)
    drv["bk3_c"] = f32(np.asarray(g["bk3"], np.float32).reshape(2, 128).T)
    for nm in ("bv1", "bv2", "bv3", "blv"):
        drv[nm + "_r"] = f32(g[nm]).reshape(1, D)
    for nm in ("bo1", "bo2", "bo3"):
        drv[nm + "_r"] = h16(np.asarray(g[nm]).reshape(1, D))
    drv["blv_b"] = h16(np.asarray(g["blv"]).reshape(1, D))
    for i in (1, 2, 3):
        drv[f"lng{i}_r"] = f32(g[f"lng{i}"]).reshape(1, D)
        drv[f"lnb{i}_r"] = f32(g[f"lnb{i}"]).reshape(1, D)
        gam = -np.logaddexp(0.0, f32(g[f"g{i}"]).reshape(H))
        drv[f"gam2_{i}"] = f32((gam * gam).reshape(1, H))
    know = f32(g["know"]).reshape(D)
    q3 = know @ f32(g["Wq3"]).T + f32(g["bq3"])
    q3blk = np.zeros((D, H), np.float32)
    for h in range(H):
        q3blk[h * DK:(h + 1) * DK, h] = q3[h * DK:(h + 1) * DK] * ISQ
    drv["q3blk"] = h16(q3blk)
    drv["know_r"] = know.reshape(1, D)
    kk = know.reshape(H, DK) @ f32(g["Wlk"]).T + f32(g["blk"])
    kk = 1.0 / (1.0 + np.exp(-kk))
    drv["kkT"] = h16(kk.T)                                # [256, 8]
    drv["WlvT"] = h16(np.tile(g["Wlv"].T, (4, 1)))        # [128, 256] x4 row groups
    p = np.arange(128)[:, None]
    j = np.arange(128)[None, :]
    pos = np.concatenate(
        [np.abs((7 - ob) * 128 + p - j).astype(np.float32) for ob in range(8)],
        axis=1)
    drv["REVPOS"] = np.ascontiguousarray(pos.astype(bf16))
    # mask constant, added to PRE-scaled scores (ISQ folded into Wq): -6e4
    # makes exp() underflow to exactly 0 and w = exp(eff * -6e4) = 0.
    drv["M0B"] = np.ascontiguousarray(
        np.where(j <= p, 0.0, -6e4).astype(bf16))         # inclusive causal
    drv["M3"] = np.ascontiguousarray(
        np.where(j < p, 0.0, -6e4).astype(bf16))          # strict causal
    drv["IDF"] = f32(np.eye(128))
    drv["IDH"] = np.ascontiguousarray(np.eye(128).astype(f16))
    drv["ONES4"] = h16(np.ones((128, 128)))
    drv["IDB"] = np.ascontiguousarray(np.eye(128).astype(bf16))
    return drv


# ---------------------------------------------------------------- builder

class KB:
    def __init__(self, nc, tc, ctx):
        self.nc, self.tc, self.ctx = nc, tc, ctx

    def pst(self, shape, dtype=F32):
        """Shared small PSUM scratch (single tag, <=512 f32 per partition)."""
        return self.pps.tile(shape, dtype, tag="ps", name="ps")

    def load_consts(self, dd):
        nc = self.nc
        pool = self.ctx.enter_context(self.tc.tile_pool(name="consts", bufs=1))
        sb = {}
        for i, names in ((1, ("q", "v", "o")), (2, ("q", "v", "o")),
                         (3, ("k", "v", "o"))):
            for n in names:
                t = pool.tile([128, 2, D], F16, tag=f"WT_{n}{i}")
                nc.sync.dma_start(
                    out=t[:],
                    in_=dd[f"WT_{n}{i}"][:].rearrange("(a p) d -> p a d", p=128))
                sb[f"WT_{n}{i}"] = t
        for nm, dt in (("q3blk", F16), ("kkT", F16)):
            t = pool.tile([128, 2, H], dt, tag=nm)
            nc.sync.dma_start(
                out=t[:], in_=dd[nm][:].rearrange("(a p) h -> p a h", p=128))
            sb[nm] = t
        for nm in ("bq1_c", "bq2_c", "bk3_c", "WlvT", "REVPOS", "M0B", "M3",
                   "IDF", "IDH", "IDB", "bo1_r", "bo2_r", "bo3_r", "blv_r",
                   "ONES4"):
            src = dd[nm]
            t = pool.tile(list(src.shape), src.dtype, tag=nm)
            nc.sync.dma_start(out=t[:], in_=src[:])
            sb[nm] = t
        for nm in ("bv1_r", "bv2_r", "bv3_r", "lng1_r", "lng2_r", "lng3_r",
                   "lnb1_r", "lnb2_r", "lnb3_r", "know_r", "gam2_1", "gam2_2",
                   "gam2_3", "blv_b"):
            src = dd[nm]
            n = src.shape[1]
            t = pool.tile([128, n], src.dtype, tag=nm)
            nc.sync.dma_start(out=t[:], in_=_prow(src, n))
            sb[nm] = t
        ones = pool.tile([1, 128], F32, tag="ones")
        nc.vector.memset(ones[:], 1.0)
        sb["ones"] = ones
        onesh = pool.tile([1, 128], F16, tag="onesh")
        nc.vector.memset(onesh[:], 1.0)
        sb["onesh"] = onesh
        onesb = pool.tile([1, 128], BF16, tag="onesb")
        nc.vector.memset(onesb[:], 1.0)
        sb["onesb"] = onesb
        epst = pool.tile([128, 1], F32, tag="eps")
        nc.vector.memset(epst[:], EPS)
        sb["eps"] = epst
        self.sb = sb
        # pre-touch identities on PE so later transposes carry a single
        # DMA-queue wait (walrus allows only one sync wait on LDWEIGHTS)
        junk = pool.tile([128, 3], F32, tag="junk")
        wf = self.pps.tile([128, 128], F32, tag="ps", name="warmf")
        nc.tensor.transpose(wf[:], sb["IDF"][:], sb["IDF"][:])
        nc.scalar.copy(out=junk[:, 0:1], in_=wf[:, 0:1])
        wb = self.pps.tile([128, 128], BF16, tag="ps", name="warmb")
        nc.tensor.transpose(wb[:], sb["IDB"][:], sb["IDB"][:])
        nc.scalar.copy(out=junk[:, 1:2], in_=wb[:, 0:1])
        wh = self.pps.tile([128, 128], F16, tag="ps", name="warmh")
        nc.tensor.transpose(wh[:], sb["IDH"][:], sb["IDH"][:])
        nc.scalar.copy(out=junk[:, 2:3], in_=wh[:, 0:1])

    def hslice(self, T, h, cols):
        """Head-rows slice of a [128, 2, SEQ] transposed tensor: [32, len]."""
        return _opt(T[(h % 4) * DK:(h % 4 + 1) * DK, h // 4, cols])

    def load_nat(self, dram, pool, tag):
        tiles = []
        for st in range(NQT):
            t = pool.tile([128, D], F32, tag=f"{tag}{st}")
            self.nc.sync.dma_start(out=t[:],
                                   in_=dram[st * 128:(st + 1) * 128, :])
            tiles.append(t)
        return tiles

    def transpose_nat(self, x_tiles, pool, tag, in_dt=F32):
        """natural [8][128,256] (tiles or APs) -> [128, 2, 1024] f16."""
        nc = self.nc
        ident = self.sb["IDF"] if in_dt == F32 else self.sb["IDH"]
        xT = pool.tile([128, 2, SEQ], F16, tag=tag)
        for st in range(NQT):
            ps = self.pst([128, 2, 128], in_dt)
            for dh in range(2):
                nc.tensor.transpose(_opt(ps[:, dh, :]),
                                    _opt(x_tiles[st][:, dh * 128:(dh + 1) * 128]),
                                    ident[:])
            nc.scalar.copy(out=_opt(xT[:, :, st * 128:(st + 1) * 128]),
                           in_=ps[:])
        return xT

    def proj_T(self, xT, wname, bname, pool, tag):
        """out[do, s] = W @ x.T + b : [128, 2, 1024] f16."""
        nc = self.nc
        W = self.sb[wname]
        out = pool.tile([128, 2, SEQ], F16, tag=tag)
        for dh in range(2):
            for sc in range(2):
                ps = self.pst([128, 512])
                for ih in range(2):
                    nc.tensor.matmul(
                        ps[:], _opt(W[:, ih, dh * 128:(dh + 1) * 128]),
                        _opt(xT[:, ih, sc * 512:(sc + 1) * 512]),
                        start=(ih == 0), stop=(ih == 1))
                nc.scalar.activation(
                    out=_opt(out[:, dh, sc * 512:(sc + 1) * 512]), in_=ps[:],
                    func=AF.Identity, bias=self.sb[bname][:, dh:dh + 1],
                    scale=1.0)
        return out

    def proj_V(self, xT, wname, bname, pool, tag):
        """V natural with ones column: [8][128, H, 33] bf16."""
        nc = self.nc
        W = self.sb[wname]
        bias = self.sb[bname]
        tiles = []
        for st in range(NQT):
            ps = self.pst([128, D])
            for ih in range(2):
                nc.tensor.matmul(ps[:],
                                 _opt(xT[:, ih, st * 128:(st + 1) * 128]),
                                 _opt(W[:, ih, :]),
                                 start=(ih == 0), stop=(ih == 1))
            v = pool.tile([128, H, DK + 1], BF16, tag=f"{tag}{st}")
            nc.vector.tensor_tensor(
                out=v[:, :, 0:DK],
                in0=ps[:].rearrange("p (h d) -> p h d", h=H),
                in1=bias[:].rearrange("p (h d) -> p h d", h=H), op=OP.add)
            nc.vector.memset(_opt(v[:, :, DK:DK + 1]), 1.0)
            tiles.append(v)
        return tiles

    # ------------------------------------------------ attention (one layer)
    def attention(self, lay, QT, KT, V, attn_pool, c3bc=None, ncum3bc=None,
                  cum3pad=None):
        nc, sb, tc = self.nc, self.sb, self.tc
        strict = (lay == 3)
        with contextlib.ExitStack() as actx:
            if not strict:
                zq = actx.enter_context(
                    tc.tile_pool(name=f"zq{lay}", bufs=2, space="PSUM"))
            oq = actx.enter_context(
                tc.tile_pool(name=f"oq{lay}", bufs=1 if not strict else 2,
                             space="PSUM"))
            rowp = actx.enter_context(tc.tile_pool(name=f"row{lay}", bufs=2))
            zsp = actx.enter_context(tc.tile_pool(name=f"zsp{lay}", bufs=3))
            chn = actx.enter_context(tc.tile_pool(name=f"chn{lay}", bufs=2))
            sml = actx.enter_context(tc.tile_pool(name=f"sml{lay}", bufs=4))
            wts = actx.enter_context(tc.tile_pool(name=f"wts{lay}", bufs=3))

            attn = []
            for qt in range(NQT):
                Kt = (qt + 1) * 128
                rp_ap = _opt(sb["REVPOS"][:, (7 - qt) * 128:
                                          (7 - qt) * 128 + Kt])
                at = attn_pool.tile([128, H, DK], F16, tag=f"attn{lay}_{qt}")
                G = {}
                for hg in range(NG):
                    g = {}
                    g["u"] = chn.tile([128, HG, Kt], BF16, tag="u", name="u")
                    g["t"] = rowp.tile([128, HG, Kt], F16, tag="t", name="t")
                    g["w"] = rowp.tile([128, HG, Kt], BF16, tag="w", name="w")
                    g["o"] = oq.tile([128, HG, DK + 1], F32, tag="o", name="o")
                    if not strict:
                        g["e"] = rowp.tile([128, HG, Kt], BF16, tag="e",
                                           name="e")
                        g["S"] = rowp.tile([128, HG, Kt], BF16, tag="S",
                                           name="S")
                        g["zs"] = zsp.tile([128, HG, Kt], F16, tag="zs",
                                           name="zs")
                        g["E"] = sml.tile([128, HG], F32, tag="E", name="E")
                    g["rEg"] = sml.tile([128, HG], F32, tag="rEg", name="rEg")
                    G[hg] = g
                if strict:
                    # E3 column for this q-tile: cum3pad[h, q] transposed
                    e3ps = self.pst([128, 8])
                    nc.tensor.transpose(
                        e3ps[:], _opt(cum3pad[:, qt * 128:qt * 128 + 128]),
                        _opt(sb["IDF"][0:8, 0:8]))
                    E3 = sml.tile([128, H], F32, tag="E3", name="E3")
                    nc.vector.tensor_scalar_max(out=E3[:], in0=e3ps[:],
                                                scalar1=1e-30)
                    rE3 = sml.tile([128, H], F32, tag="rE3", name="rE3")
                    nc.vector.reciprocal(out=rE3[:], in_=E3[:])
                    nc.vector.tensor_tensor(out=rE3[:], in0=rE3[:],
                                            in1=sb["gam2_3"][:], op=OP.mult)

                if not strict:
                    # ---- scores -> PSUM (f16 inputs, ISQ pre-folded into Wq)
                    for hg in range(NG):
                        g = G[hg]
                        for i, h in enumerate(range(hg * HG, hg * HG + HG)):
                            z = zq.tile([128, Kt], F32, tag="z", name="z")
                            lhs = self.hslice(
                                QT, h, slice(qt * 128, qt * 128 + 128))
                            tp = ((h % 4) * DK, 0)
                            nchunk = (Kt + 511) // 512
                            for ci in range(nchunk):
                                kc = ci * 512
                                cl = min(512, Kt - kc)
                                nc.tensor.matmul(
                                    _opt(z[:, kc:kc + cl]), lhs,
                                    self.hslice(KT, h, slice(kc, kc + cl)),
                                    start=True, stop=False, tile_position=tp,
                                    skip_group_check=True)
                            nc.tensor.matmul(
                                _opt(z[:, Kt - 128:Kt]), sb["IDB"][:],
                                sb["M0B"][:], start=False, stop=True,
                                skip_group_check=True)
                            # zs: cast copy to f16 (V/S alternate)
                            if i % 2 == 0:
                                nc.vector.tensor_copy(
                                    out=_opt(g["zs"][:, i, :]), in_=z[:])
                            else:
                                nc.scalar.copy(
                                    out=_opt(g["zs"][:, i, :]), in_=z[:])
                    # ---- e = exp(zs)
                    for hg in range(NG):
                        nc.scalar.activation(out=G[hg]["e"][:],
                                             in_=G[hg]["zs"][:], func=AF.Exp)
                    # ---- reversed scan -> exclusive suffix sums S
                    for hg in range(NG):
                        g = G[hg]
                        for i in range(HG):
                            nc.vector.tensor_tensor_scan(
                                out=_rev(g["S"][:, i, 0:Kt - 1]),
                                data0=_rev(g["e"][:, i, 1:Kt]),
                                data1=_rev(g["e"][:, i, 1:Kt]),
                                initial=0.0, op0=OP.add, op1=OP.bypass)
                            nc.vector.memset(_opt(g["S"][:, i, Kt - 1:Kt]),
                                             0.0)
                    # ---- E = S[0] + e[0];  rEg = gamma^2 / E
                    for hg in range(NG):
                        g = G[hg]
                        nc.vector.tensor_tensor(
                            out=g["E"][:], in0=_opt(g["S"][:, :, 0:1]),
                            in1=_opt(g["e"][:, :, 0:1]), op=OP.add)
                        nc.vector.reciprocal(out=g["rEg"][:], in_=g["E"][:])
                        nc.vector.tensor_tensor(
                            out=g["rEg"][:], in0=g["rEg"][:],
                            in1=_opt(sb[f"gam2_{lay}"][:, hg * HG:
                                                       hg * HG + HG]),
                            op=OP.mult)
                    # ---- u = S * pos  (2x TT; rEg folded into sqrt scale)
                    for hg in range(NG):
                        g = G[hg]
                        nc.vector.tensor_tensor(
                            out=g["u"][:], in0=g["S"][:],
                            in1=_bc_mid(rp_ap, HG), op=OP.mult)
                    # ---- d = sqrt(u * rEg_h) per head; eff = exp(-d)
                    for hg in range(NG):
                        g = G[hg]
                        for i in range(HG):
                            nc.scalar.activation(
                                out=_opt(g["u"][:, i, :]),
                                in_=_opt(g["u"][:, i, :]),
                                func=AF.Sqrt,
                                scale=_opt(g["rEg"][:, i:i + 1]))
                    for hg in range(NG):
                        nc.scalar.activation(out=G[hg]["u"][:],
                                             in_=G[hg]["u"][:], func=AF.Exp,
                                             scale=-1.0)
                    # ---- t = eff * zs  (2x TT, clip dropped: irrelevant at
                    # rel-tol 2e-2 since eff<1e-5 implies |t|<3e-4)
                    for hg in range(NG):
                        g = G[hg]
                        nc.vector.tensor_tensor(
                            out=g["t"][:], in0=g["u"][:], in1=g["zs"][:],
                            op=OP.mult)
                else:
                    # ---- layer 3: u = (E3 + (-cum)) * pos, clamp diag
                    for hg in range(NG):
                        g = G[hg]
                        for i, h in enumerate(range(hg * HG, hg * HG + HG)):
                            nc.vector.scalar_tensor_tensor(
                                out=_opt(g["u"][:, i, :]),
                                in0=_opt(ncum3bc[h][:, 0:Kt]),
                                scalar=_opt(E3[:, h:h + 1]),
                                in1=rp_ap, op0=OP.add, op1=OP.mult)
                            nc.vector.tensor_scalar_max(
                                out=_opt(g["u"][:, i, Kt - 128:Kt]),
                                in0=_opt(g["u"][:, i, Kt - 128:Kt]),
                                scalar1=0.0)
                    # ---- d = sqrt(u * gamma^2/E) per head (scale AP)
                    for hg in range(NG):
                        g = G[hg]
                        for i, h in enumerate(range(hg * HG, hg * HG + HG)):
                            nc.scalar.activation(
                                out=_opt(g["u"][:, i, :]),
                                in_=_opt(g["u"][:, i, :]),
                                func=AF.Sqrt, scale=_opt(rE3[:, h:h + 1]))
                    for hg in range(NG):
                        nc.scalar.activation(out=G[hg]["u"][:],
                                             in_=G[hg]["u"][:], func=AF.Exp,
                                             scale=-1.0)
                    # ---- t = eff * c3 (masked diag; clip dropped)
                    for hg in range(NG):
                        g = G[hg]
                        for i, h in enumerate(range(hg * HG, hg * HG + HG)):
                            if qt > 0:
                                nc.vector.tensor_tensor(
                                    out=_opt(g["t"][:, i, 0:Kt - 128]),
                                    in0=_opt(g["u"][:, i, 0:Kt - 128]),
                                    in1=_opt(c3bc[h][:, 0:Kt - 128]),
                                    op=OP.mult)
                            c3m = sml.tile([128, 128], BF16, tag="c3m",
                                           name="c3m")
                            nc.vector.tensor_tensor(
                                out=c3m[:], in0=_opt(c3bc[h][:, Kt - 128:Kt]),
                                in1=sb["M3"][:], op=OP.add)
                            nc.vector.tensor_tensor(
                                out=_opt(g["t"][:, i, Kt - 128:Kt]),
                                in0=_opt(g["u"][:, i, Kt - 128:Kt]),
                                in1=c3m[:], op=OP.mult)
                # ---- w = exp(t)
                for hg in range(NG):
                    nc.scalar.activation(out=G[hg]["w"][:], in_=G[hg]["t"][:],
                                         func=AF.Exp)
                # ---- transpose w via DMA xbar; A @ [V | 1] accumulation
                for hg in range(NG):
                    g = G[hg]
                    for i, h in enumerate(range(hg * HG, hg * HG + HG)):
                        wT = wts.tile([128, qt + 1, 128], BF16, tag="wT",
                                      name="wT")
                        nc.sync.dma_start(out=wT[:], in_=_opt(g["w"][:, i, :]),
                                          transpose=True)
                        for kb in range(qt + 1):
                            nc.tensor.matmul(
                                _opt(g["o"][:, i, :]),
                                _opt(wT[:, kb, :]),
                                _opt(V[kb][:, h, :]),
                                start=(kb == 0), stop=(kb == qt),
                                skip_group_check=True)
                # ---- normalize attn = o / W
                for hg in range(NG):
                    g = G[hg]
                    Wg = sml.tile([128, HG], F32, tag="Wg", name="Wg")
                    rW = sml.tile([128, HG], F32, tag="rW", name="rW")
                    nc.vector.tensor_scalar_max(
                        out=Wg[:], in0=_opt(g["o"][:, :, DK:DK + 1]),
                        scalar1=1e-30)
                    nc.vector.reciprocal(out=rW[:], in_=Wg[:])
                    nc.vector.tensor_tensor(
                        out=_opt(at[:, hg * HG:hg * HG + HG, :]),
                        in0=_opt(g["o"][:, :, 0:DK]), in1=_bc(rW[:], DK),
                        op=OP.mult)
                attn.append(at)
            return attn

    # ------------------------------------------------ out proj + LN
    def out_ln(self, lay, attn, res_tiles, hpool, spool, tpool,
               dram_out=None):
        nc, sb = self.nc, self.sb
        attnT = self.transpose_nat(
            [a[:].rearrange("p h d -> p (h d)") for a in attn],
            tpool, tag=f"attnT{lay}", in_dt=F16)
        W = sb[f"WT_o{lay}"]
        out_tiles = []
        for st in range(NQT):
            ps = self.pst([128, D])
            for ih in range(2):
                nc.tensor.matmul(ps[:],
                                 _opt(attnT[:, ih, st * 128:(st + 1) * 128]),
                                 _opt(W[:, ih, :]), start=(ih == 0), stop=False)
            nc.tensor.matmul(ps[:], sb["onesh"][:], sb[f"bo{lay}_r"][:],
                             start=False, stop=True)
            res = res_tiles[st] if isinstance(res_tiles, list) else res_tiles
            x = spool.tile([128, D], F32, tag="lnx")
            nc.vector.tensor_tensor(out=x[:], in0=ps[:], in1=res[:], op=OP.add)
            stats = spool.tile([128, 6], F32, tag="bnst")
            mv = spool.tile([128, 2], F32, tag="bnmv")
            nc.vector.bn_stats(out=stats[:], in_=x[:])
            nc.vector.bn_aggr(out=mv[:], in_=stats[:])
            sd = spool.tile([128, 1], F32, tag="sd")
            nc.scalar.activation(out=sd[:], in_=_opt(mv[:, 1:2]), func=AF.Sqrt,
                                 bias=sb["eps"][:], scale=1.0)
            rstd = spool.tile([128, 1], F32, tag="rstd")
            nc.vector.reciprocal(out=rstd[:], in_=sd[:])
            xn = spool.tile([128, D], F32, tag="lnxn")
            nc.vector.tensor_scalar(
                out=xn[:], in0=x[:], scalar1=_opt(mv[:, 0:1]), scalar2=rstd[:],
                op0=OP.subtract, op1=OP.mult)
            if dram_out is None:
                ho = hpool.tile([128, D], F32, tag=f"h{lay}_{st}")
            else:
                ho = spool.tile([128, D], F32, tag="ho", name="ho")
            nc.vector.tensor_tensor(out=ho[:], in0=xn[:],
                                    in1=sb[f"lng{lay}_r"][:], op=OP.mult)
            nc.vector.tensor_tensor(out=ho[:], in0=ho[:],
                                    in1=sb[f"lnb{lay}_r"][:], op=OP.add)
            if dram_out is not None:
                nc.sync.dma_start(out=dram_out[st * 128:(st + 1) * 128, :],
                                  in_=ho[:])
            out_tiles.append(ho)
        return out_tiles

    # ------------------------------------------------ layer-3 prologue
    def l3_rows(self, h1T, mpool, lpool):
        nc, sb = self.nc, self.sb
        KT3 = self.proj_T(h1T, "WT_k3", "bk3_c", mpool, tag="KT3")
        c3 = lpool.tile([8, SEQ], F32, tag="c3")
        for sc in range(2):
            ps = self.pst([8, 512])
            for ih in range(2):
                nc.tensor.matmul(ps[:], _opt(sb["q3blk"][:, ih, :]),
                                 _opt(KT3[:, ih, sc * 512:(sc + 1) * 512]),
                                 start=(ih == 0), stop=(ih == 1))
            nc.vector.tensor_copy(
                out=_opt(c3[:, sc * 512:(sc + 1) * 512]), in_=ps[:])
        e3 = lpool.tile([8, SEQ], F32, tag="e3")
        nc.scalar.activation(out=e3[:], in_=c3[:], func=AF.Exp)
        cum3pad = lpool.tile([8, SEQ + 128], F32, tag="cum3pad")
        nc.vector.memset(_opt(cum3pad[:, 0:1]), 0.0)
        nc.vector.tensor_tensor_scan(
            out=_opt(cum3pad[:, 1:SEQ + 1]), data0=e3[:], data1=e3[:],
            initial=0.0, op0=OP.add, op1=OP.bypass)
        nc.vector.memset(_opt(cum3pad[:, SEQ + 1:]), 0.0)
        # bf16 copy of c3 rows (safe: consumed as bf16 anyway)
        c3b = lpool.tile([8, SEQ], BF16, tag="c3b")
        nc.vector.tensor_copy(out=c3b[:], in_=c3[:])
        # negated cum rows (f32: cancellation-sensitive)
        ncum = lpool.tile([8, SEQ], F32, tag="ncum")
        nc.vector.tensor_scalar_mul(
            out=ncum[:], in0=_opt(cum3pad[:, 1:SEQ + 1]), scalar1=-1.0)
        # round-trip the rows through DRAM, then DMA-replicate across
        # partitions (same pattern as the const-row broadcasts)
        c3d = nc.dram_tensor("c3rows", [8, SEQ], BF16, kind="Internal")
        ncumd = nc.dram_tensor("ncumrows", [8, SEQ], F32, kind="Internal")
        nc.sync.dma_start(out=c3d[:, :], in_=c3b[:])
        nc.sync.dma_start(out=ncumd[:, :], in_=ncum[:])
        c3bc, ncum3bc = [], []
        for h in range(H):
            cb = lpool.tile([128, SEQ], BF16, tag=f"c3bc{h}")
            ncb = lpool.tile([128, SEQ], F32, tag=f"ncum{h}")
            nc.scalar.dma_start(
                out=cb[:], in_=bass.AP(tensor=c3d, offset=h * SEQ,
                                       ap=[[0, 128], [1, SEQ]]))
            nc.scalar.dma_start(
                out=ncb[:], in_=bass.AP(tensor=ncumd, offset=h * SEQ,
                                        ap=[[0, 128], [1, SEQ]]))
            c3bc.append(cb)
            ncum3bc.append(ncb)
        return c3bc, ncum3bc, cum3pad

    # ------------------------------------------------ final mixture
    def final(self, hh, xT_q, out_dram, spool, tpool):
        nc, sb = self.nc, self.sb
        hhT = self.transpose_nat(hh, tpool, tag="hhT")
        als = []
        for st in range(NQT):
            bps = self.pst([128, H])
            for ih in range(2):
                nc.tensor.matmul(bps[:],
                                 _opt(xT_q[:, ih, st * 128:(st + 1) * 128]),
                                 _opt(sb["kkT"][:, ih, :]),
                                 start=(ih == 0), stop=(ih == 1))
            nmax = spool.tile([128, 1], F32, tag="nmax", name="nmax")
            nc.vector.tensor_reduce(out=nmax[:], in_=bps[:],
                                    axis=mybir.AxisListType.X, op=OP.max,
                                    negate=True)
            au = spool.tile([128, H], F32, tag="au", name="au")
            sa = spool.tile([128, 1], F32, tag="sa", name="sa")
            nc.scalar.activation(out=au[:], in_=bps[:], func=AF.Exp,
                                 bias=nmax[:], scale=1.0, accum_out=sa[:])
            rsa = spool.tile([128, 1], F32, tag="rsa", name="rsa")
            nc.vector.reciprocal(out=rsa[:], in_=sa[:])
            al = tpool.tile([128, H], F32, tag=f"al{st}", name="al")
            nc.vector.tensor_scalar_mul(out=al[:], in0=au[:], scalar1=rsa[:])
            als.append(al)
        for st in range(NQT):
            al = als[st]
            acc = spool.tile([128, D], F32, tag="facc", name="facc")
            for h in range(H):
                vps = self.pst([128, D])
                nc.tensor.matmul(
                    vps[:],
                    self.hslice(hhT, h, slice(st * 128, st * 128 + 128)),
                    _opt(sb["WlvT"][(h % 4) * DK:(h % 4 + 1) * DK, :]),
                    start=True, stop=False,
                    tile_position=((h % 4) * DK, 0), skip_group_check=True)
                o = (h % 4) * DK
                nc.tensor.matmul(
                    vps[:], _opt(sb["ONES4"][o:o + 1, :]),
                    _opt(sb["blv_b"][o:o + 1, :]), start=False, stop=True,
                    tile_position=(o, 0), skip_group_check=True)
                vsb = spool.tile([128, D], BF16, tag="vsb", name="vsb")
                nc.scalar.activation(out=vsb[:], in_=vps[:], func=AF.Sigmoid)
                if h == 0:
                    nc.vector.tensor_scalar_mul(out=acc[:], in0=vsb[:],
                                                scalar1=al[:, 0:1])
                else:
                    nc.vector.scalar_tensor_tensor(
                        out=acc[:], in0=vsb[:], scalar=al[:, h:h + 1],
                        in1=acc[:], op0=OP.mult, op1=OP.add)
            nc.sync.dma_start(out=out_dram[st * 128:(st + 1) * 128, :],
                              in_=acc[:])


def build(derived, debug=False, stop_after=None):
    nc = bacc.Bacc(None, target_bir_lowering=False)
    dd = {}
    for name, arr in derived.items():
        dt = {np.dtype(np.float32): F32, np.dtype(bf16): BF16,
              np.dtype(f16): F16}[np.dtype(arr.dtype)]
        dd[name] = nc.dram_tensor(name, list(arr.shape), dt,
                                  kind="ExternalInput")
    x_q = nc.dram_tensor("x_q", [SEQ, D], F32, kind="ExternalInput")
    x_s = nc.dram_tensor("x_s", [SEQ, D], F32, kind="ExternalInput")
    out = nc.dram_tensor("out", [SEQ, D], F32, kind="ExternalOutput")

    def dump(tiles, name):
        if not debug:
            return
        dt = tiles[0][:].dtype
        t = nc.dram_tensor(name, [SEQ, D], dt, kind="ExternalOutput")
        for st in range(NQT):
            ap = tiles[st][:]
            if len(ap.shape) == 3:
                ap = ap.rearrange("p h d -> p (h d)")
            nc.sync.dma_start(out=t[st * 128:(st + 1) * 128, :], in_=ap)

    with tile.TileContext(nc) as tc, contextlib.ExitStack() as ctx:
        kb = KB(nc, tc, ctx)
        kb.pps = ctx.enter_context(
            tc.tile_pool(name="pps", bufs=2, space="PSUM"))
        kb.load_consts(dd)
        sb = kb.sb
        glob = ctx.enter_context(tc.tile_pool(name="glob", bufs=1))

        h1d = nc.dram_tensor("dbg_h1", [SEQ, D], F32,
                             kind="ExternalOutput" if debug else "Internal")
        h2d = nc.dram_tensor("dbg_h2", [SEQ, D], F32,
                             kind="ExternalOutput" if debug else "Internal")
        # ---------------- layer 1 (on x_q) ----------------
        with tc.tile_pool(name="r1", bufs=1) as r1, \
                tc.tile_pool(name="r1s", bufs=2) as r1s:
            xq_nat = kb.load_nat(x_q, r1, "xq")
            xT_q = kb.transpose_nat(xq_nat, glob, tag="xTq")
            QT1 = kb.proj_T(xT_q, "WT_q1", "bq1_c", r1, tag="QT1")
            V1 = kb.proj_V(xT_q, "WT_v1", "bv1_r", r1, tag="V1")
            if stop_after == "pre1":
                for st in range(NQT):
                    nc.sync.dma_start(out=out[st * 128:(st + 1) * 128, :],
                                      in_=xq_nat[st][:])
            if stop_after in (None, "l1", "l2", "pro", "l3"):
                attn1 = kb.attention(1, QT1, QT1, V1, r1)
                dump(attn1, "dbg_attn1")
                kb.out_ln(1, attn1, xq_nat, None, r1s, r1, dram_out=h1d)
            if stop_after == "l1":
                h1n0 = kb.load_nat(h1d, r1, "h1o")
                for st in range(NQT):
                    nc.sync.dma_start(out=out[st * 128:(st + 1) * 128, :],
                                      in_=h1n0[st][:])
        # ---------------- layer 2 (on x_s) ----------------
        if stop_after in (None, "l2", "pro", "l3"):
            with tc.tile_pool(name="r2", bufs=1) as r2, \
                    tc.tile_pool(name="r2s", bufs=2) as r2s:
                xs_nat = kb.load_nat(x_s, r2, "xs")
                xT_s = kb.transpose_nat(xs_nat, r2, tag="xTs")
                QT2 = kb.proj_T(xT_s, "WT_q2", "bq2_c", r2, tag="QT2")
                V2 = kb.proj_V(xT_s, "WT_v2", "bv2_r", r2, tag="V2")
                attn2 = kb.attention(2, QT2, QT2, V2, r2)
                kb.out_ln(2, attn2, xs_nat, None, r2s, r2, dram_out=h2d)
                if stop_after == "l2":
                    h2n0 = kb.load_nat(h2d, r2, "h2o")
                    for st in range(NQT):
                        nc.sync.dma_start(out=out[st * 128:(st + 1) * 128, :],
                                          in_=h2n0[st][:])
        # ---------------- layer 3 prologue ----------------
        if stop_after in (None, "pro", "l3"):
            lpool = ctx.enter_context(tc.tile_pool(name="l3pool", bufs=1))
            with tc.tile_pool(name="l3tmp", bufs=1) as l3tmp:
                h1n = kb.load_nat(h1d, l3tmp, "h1n")
                h2n = kb.load_nat(h2d, l3tmp, "h2n")
                h1T = kb.transpose_nat(h1n, l3tmp, tag="h1T")
                h2T = kb.transpose_nat(h2n, l3tmp, tag="h2T")
                V3 = kb.proj_V(h2T, "WT_v3", "bv3_r", lpool, tag="V3")
                c3bc, ncum3bc, cum3pad = kb.l3_rows(h1T, l3tmp, lpool)
                if stop_after == "pro":
                    for st in range(NQT):
                        o32 = l3tmp.tile([128, D], F32, tag=f"o32_{st}")
                        nc.vector.tensor_copy(
                            out=o32[:].rearrange("p (h d) -> p h d", h=H),
                            in_=V3[st][:, :, 0:DK])
                        nc.sync.dma_start(out=out[st * 128:(st + 1) * 128, :],
                                          in_=o32[:])
        # ---------------- layer 3 + final ----------------
        if stop_after in (None, "l3"):
            with tc.tile_pool(name="r3", bufs=1) as r3, \
                    tc.tile_pool(name="r3s", bufs=2) as r3s:
                attn3 = kb.attention(3, None, None, V3, r3, c3bc=c3bc,
                                     ncum3bc=ncum3bc, cum3pad=cum3pad)
                dump(attn3, "dbg_attn3")
                hh = kb.out_ln(3, attn3, sb["know_r"], r3, r3s, r3)
                dump(hh, "dbg_hh")
                if stop_after == "l3":
                    for st in range(NQT):
                        nc.sync.dma_start(out=out[st * 128:(st + 1) * 128, :],
                                          in_=hh[st][:])
                if stop_after is None:
                    kb.final(hh, xT_q, out, r3s, r3)
    nc.compile()
    return nc


_CACHE = {}


def kernel(**inputs):
    drv = host_prep(inputs)
    if "nc" not in _CACHE:
        _CACHE["nc"] = build(drv)
    nc = _CACHE["nc"]
    q = np.ascontiguousarray(np.asarray(inputs["q_emb"], np.float32))
    s = np.ascontiguousarray(np.asarray(inputs["s_emb"], np.float32))
    in_maps = []
    for b in range(BS):
        m = dict(drv)
        m["x_q"] = np.ascontiguousarray(q[b])
        m["x_s"] = np.ascontiguousarray(s[b])
        in_maps.append(m)
    from concourse.bass_utils import run_bass_kernel_spmd
    res = run_bass_kernel_spmd(nc, in_maps, core_ids=list(range(BS)))
    out = np.stack([np.asarray(res.results[b]["out"]) for b in range(BS)],
                   axis=0)
    return out.astype(np.float32)


if __name__ == "__main__":
    print("kernel module loaded OK")
